# revision 6
# baseline (speedup 1.0000x reference)
"""DiceLoss kernel for Trainium2 (Bass/Tile), data-parallel over batch on 8 cores.

Problem: inputs [8, 21, 512, 512] f32 logits, targets [8, 512, 512] int64,
smooth scalar. reference = mean_b dice_b with
  dice_b = 1 - (2*I_b + s) / (S_b + T_b + s)
where probs = softmax(inputs, axis=1),
  I_b = sum_pix probs[target]        (ignore_index=255 pixels excluded)
  S_b = sum probs * mask = sum mask  (softmax sums to 1 over classes)
  T_b = sum mask.

Device kernel (per core = one batch element):
  For each class c: e_c = exp(x_c)  (no max-subtraction needed: |x| <~ 5.5)
    m_c = (t == c)  -> one-hot;  gm_c = e_c * m_c
    PSUM z += e_c   (identity-matmul accumulate on the tensor engine)
    PSUM g += gm_c
  r = 1/z (approx);  I = sum(g * r);  N = sum(t != 255)
  out = [I_per_partition, N_per_partition]  (host sums the 128 partials)

Everything is bf16 on the wire/compute except PSUM/f32 accumulators: the
bf16 quantization of e appears in both numerator and denominator of the
softmax ratio and largely cancels; residual ~0.3%/pixel random error
averages out over 262144 pixels (final rel err ~1e-5).
"""

import numpy as np
import ml_dtypes

B, C, H, W = 8, 21, 512, 512
HW = H * W           # 262144
P = 128              # SBUF partitions
FREE = HW // P       # 2048 free-dim elements per partition
N_CORES = 8
MM_N = 512           # matmul free-dim chunk (one PSUM bank of f32)

_STATE = {}


def _patch_tile_drain():
    """This neuronxcc build rejects >1 sync-wait per instruction ("Too many
    sync wait commands"). Split multi-wait instructions: hoist extra waits
    onto single-wait InstNoOps inserted just before, on the same engine."""
    import concourse.tile as tile
    from concourse.vector_clock import ScopedClock
    from concourse import mybir
    import bass_rust

    if getattr(tile.TileContext, "_ant_drain_patched", False):
        return

    _orig_lower = tile.TileContext._lower_ordered_insts

    def _lower_split(self, ordered):
        for insts in ordered.values():
            new = []
            for inst in insts:
                si = getattr(inst, "sync_info", None)
                eng = getattr(inst, "engine", None)
                if (
                    si is not None
                    and si.on_wait
                    and len(si.on_wait) > 1
                    and eng is not None
                    and eng != mybir.EngineType.Unassigned
                ):
                    waits = list(si.on_wait)
                    for w in waits[:-1]:
                        new.append(
                            mybir.InstDrain(
                                name=self.nc.get_next_instruction_name(),
                                opcode="Drain",
                                engine=eng,
                                bass_is_fusable=False,
                                sync_info=bass_rust.SyncInfo(
                                    on_wait=[w], on_update=[]
                                ),
                            )
                        )
                    inst.sync_info = bass_rust.SyncInfo(
                        on_wait=[waits[-1]], on_update=list(si.on_update or [])
                    )
                new.append(inst)
            insts[:] = new
        return _orig_lower(self, ordered)

    tile.TileContext._lower_ordered_insts = _lower_split

    def _drain_and_barrier(self, tick_clock, wait_clock):
        drain_inst = self.nc.sync.drain()
        wait_clock.add_sem_waits(
            drain_inst.ins, ScopedClock({None: tick_clock.global_clock})
        )
        ins = drain_inst.ins
        si = ins.sync_info
        if si is not None and si.on_wait and len(si.on_wait) > 1:
            waits = list(si.on_wait)
            ins.sync_info = bass_rust.SyncInfo(
                on_wait=waits[:1], on_update=list(si.on_update or [])
            )
            for w in waits[1:]:
                extra = self.nc.sync.drain()
                extra.ins.sync_info = bass_rust.SyncInfo(on_wait=[w], on_update=[])
        self.nc.all_engine_barrier()
        assert self.sems is not None
        popped = self.nc._tile_sem_poison_stack.pop()
        assert popped is self._sem_poison
        self.nc.clear_and_free_semaphores(list(self.sems.allocated().values()))
        self.nc.all_engine_barrier()

    tile.TileContext._drain_and_barrier = _drain_and_barrier
    tile.TileContext._ant_drain_patched = True


def _build_nc():
    import concourse.bass as bass
    import concourse.tile as tile
    from concourse import mybir

    _patch_tile_drain()

    bf16 = mybir.dt.bfloat16
    f32 = mybir.dt.float32
    Alu = mybir.AluOpType
    Act = mybir.ActivationFunctionType

    nc = bass.Bass()
    x_d = nc.declare_dram_parameter("x", [C, P, FREE], bf16, isOutput=False)
    t_d = nc.declare_dram_parameter("t", [P, FREE], bf16, isOutput=False)
    o_d = nc.declare_dram_parameter("out", [P, 2], f32, isOutput=True)
    ident_d = nc.inline_tensor(np.eye(P, dtype=ml_dtypes.bfloat16), name="ident")

    with tile.TileContext(nc) as tc:
        with (
            tc.tile_pool(name="const", bufs=1) as constp,
            tc.tile_pool(name="xp", bufs=3) as xp,
            tc.tile_pool(name="ep", bufs=3) as ep,
            tc.tile_pool(name="mp", bufs=3) as mp,
            tc.tile_pool(name="gmp", bufs=3) as gmp,
            tc.tile_pool(name="misc", bufs=1) as misc,
            tc.tile_pool(name="psum", bufs=1, space=bass.MemorySpace.PSUM) as psp,
        ):
            ident = constp.tile([P, P], bf16)
            nc.sync.dma_start(ident[:], ident_d[:])
            t_sb = misc.tile([P, FREE], bf16)
            nc.sync.dma_start(t_sb[:], t_d[:])

            zp = psp.tile([P, FREE], f32)  # 4 PSUM banks
            gp = psp.tile([P, FREE], f32)  # 4 PSUM banks

            for c in range(C):
                xt = xp.tile([P, FREE], bf16)
                nc.sync.dma_start(xt[:], x_d[c])
                e = ep.tile([P, FREE], bf16)
                nc.scalar.activation(e[:], xt[:], Act.Exp)
                m = mp.tile([P, FREE], bf16)
                nc.vector.tensor_scalar(m[:], t_sb[:], float(c), None, Alu.is_equal)
                gm = gmp.tile([P, FREE], bf16)
                nc.vector.tensor_mul(gm[:], e[:], m[:])
                first, last = (c == 0), (c == C - 1)
                for k in range(FREE // MM_N):
                    sl = bass.ts(k, MM_N)
                    nc.tensor.matmul(
                        zp[:, sl], ident[:], e[:, sl], start=first, stop=last
                    )
                    nc.tensor.matmul(
                        gp[:, sl], ident[:], gm[:, sl], start=first, stop=last
                    )

            # r = 1/z via exp(-log(z)) on the scalar engine (the custom-DVE
            # reciprocal ops don't encode on this walrus build, and vector
            # reciprocal costs 6 cycles/elem on the busiest engine).
            u = misc.tile([P, FREE], f32)
            nc.scalar.activation(u[:], zp[:], Act.Ln)
            r = misc.tile([P, FREE], f32)
            nc.scalar.activation(r[:], u[:], Act.Exp, scale=-1.0)
            gr = misc.tile([P, FREE], f32)
            nc.vector.tensor_mul(gr[:], gp[:], r[:])
            iacc = misc.tile([P, 1], f32)
            nc.vector.reduce_sum(iacc[:], gr[:], axis=mybir.AxisListType.X)
            m255 = misc.tile([P, FREE], bf16)
            nc.vector.tensor_scalar(m255[:], t_sb[:], 255.0, None, Alu.not_equal)
            nacc = misc.tile([P, 1], f32)
            nc.vector.reduce_sum(nacc[:], m255[:], axis=mybir.AxisListType.X)

            outt = misc.tile([P, 2], f32)
            nc.vector.tensor_copy(outt[:, 0:1], iacc[:])
            nc.vector.tensor_copy(outt[:, 1:2], nacc[:])
            nc.sync.dma_start(o_d[:], outt[:])

    return nc


def _build_runner():
    """Compile once; return fn(per_core_inputs) -> list of out arrays.

    Adapted from concourse.bass2jax.run_bass_via_pjrt, but caches the jitted
    executable so repeat kernel() calls don't recompile."""
    import jax
    import jax.numpy as jnp
    from jax.sharding import Mesh, PartitionSpec
    from jax.experimental.shard_map import shard_map
    from concourse import bass2jax, mybir

    nc = _build_nc()
    bass2jax.install_neuronx_cc_hook()

    partition_name = nc.partition_id_tensor.name if nc.partition_id_tensor else None
    in_names = []
    out_names = []
    out_avals = []
    zero_outs = []
    for alloc in nc.m.functions[0].allocations:
        if not isinstance(alloc, mybir.MemoryLocationSet):
            continue
        name = alloc.memorylocations[0].name
        if alloc.kind == "ExternalInput":
            if name != partition_name:
                in_names.append(name)
        elif alloc.kind == "ExternalOutput":
            out_names.append(name)
            shape = tuple(alloc.tensor_shape)
            dtype = mybir.dt.np(alloc.dtype)
            out_avals.append(jax.core.ShapedArray(shape, dtype))
            zero_outs.append(np.zeros(shape, dtype))
    n_params = len(in_names)
    n_outs = len(out_avals)
    all_in_names = in_names + out_names
    if partition_name is not None:
        all_in_names = all_in_names + [partition_name]

    def _body(*args):
        operands = list(args)
        if partition_name is not None:
            operands.append(bass2jax.partition_id_tensor())
        outs = bass2jax._bass_exec_p.bind(
            *operands,
            out_avals=tuple(out_avals),
            in_names=tuple(all_in_names),
            out_names=tuple(out_names),
            lowering_input_output_aliases=(),
            sim_require_finite=True,
            sim_require_nnan=True,
            nc=nc,
        )
        return tuple(outs)

    devices = jax.devices()[:N_CORES]
    mesh = Mesh(np.asarray(devices), ("core",))
    in_specs = (PartitionSpec("core"),) * (n_params + n_outs)
    out_specs = (PartitionSpec("core"),) * n_outs
    donate = tuple(range(n_params, n_params + n_outs))
    sharded = jax.jit(
        shard_map(
            _body, mesh=mesh, in_specs=in_specs, out_specs=out_specs, check_rep=False
        ),
        donate_argnums=donate,
        keep_unused=True,
    )

    def run(per_core_in_maps):
        concat_in = [
            np.concatenate([m[name] for m in per_core_in_maps], axis=0)
            for name in in_names
        ]
        concat_zeros = [
            np.zeros((N_CORES * z.shape[0], *z.shape[1:]), z.dtype) for z in zero_outs
        ]
        out_arrs = sharded(*concat_in, *concat_zeros)
        return [
            np.asarray(out_arrs[0]).reshape(N_CORES, *out_avals[0].shape)[c]
            for c in range(N_CORES)
        ]

    return run


def _get_runner():
    if "runner" not in _STATE:
        _STATE["runner"] = _build_runner()
    return _STATE["runner"]


def kernel(inputs, targets, smooth):
    inputs = np.asarray(inputs)
    targets = np.asarray(targets)
    s = float(np.asarray(smooth))

    x = inputs.reshape(B, C, P, FREE).astype(ml_dtypes.bfloat16)
    t = targets.reshape(B, P, FREE).astype(ml_dtypes.bfloat16)

    in_maps = [{"x": x[b], "t": t[b]} for b in range(B)]
    run = _get_runner()
    outs = run(in_maps)

    dices = []
    for b in range(B):
        ob = outs[b].astype(np.float64)
        I_b = ob[:, 0].sum()
        N_b = ob[:, 1].sum()
        dices.append(1.0 - (2.0 * I_b + s) / (2.0 * N_b + s))
    return np.float32(np.mean(dices))
